# revision 35
# baseline (speedup 1.0000x reference)
"""Trainium2 Bass kernel for nn_AttentionBlock (GroupNorm + 4-head attention + proj + residual).

Sharding: data-parallel over batch B=16 across 8 cores (2 batches/core).
Layouts per batch (C=512 -> 4 partition tiles of 128, N=H*W=1024):
  x, h, q, k:  [128, 4(co), 1024]   channel c = co*128 + p
  vT:          [128, 8(nc), 512]    v transposed -> [n, c]; bf16
  P^T (probs): [128, 8(mc), 1024]   exp(scores^T) per head; bf16
  attn out:    [128, 4(head), 1024] normalized attention output; bf16
Scores are computed transposed (S^T[m, n]) so the PV matmul needs no transposes.
Softmax skips max-subtraction (scores bounded ~ +-7.4 for this distribution).
Denominator: DVE reduce over chunk dim + ones-matmul across partitions +
gpsimd partition_broadcast.
"""

import os

import numpy as np
import ml_dtypes

import concourse.bass as bass
import concourse.tile as tile
from concourse import mybir

B = 16
N_CORES = 8
B_LOC = B // N_CORES  # 2
C = 512
HW = 32
N = HW * HW  # 1024
NH = 4  # heads
CH = C // NH  # 128 channels/head
CO = C // 128  # 4 partition tiles over channels
NG = 8  # groups
EPS = 1e-5
SCALE = 1.0 / np.sqrt(CH)

F32 = mybir.dt.float32
BF16 = mybir.dt.bfloat16

_BUILT = None  # cached (nc,)

# Walrus in this toolchain rejects instructions carrying more than a couple of
# embedded sync waits ("Too many sync wait commands"). The Tile end-of-kernel
# drain collects one wait per live proc (11 here). Split them across several
# drain instructions on the sync engine (program order preserves semantics).
_DRAIN_WAIT_LIMIT = 1


def _patch_tile_drain():
    if getattr(tile.TileContext, "_drain_split_patched", False):
        return
    from concourse.vector_clock import ScopedClock

    orig_lower = tile.TileContext._lower_ordered_insts

    def _lower_ordered_insts(self, ordered):
        counter = [0]
        for bbname in list(ordered.keys()):
            insts = ordered[bbname]
            new = []
            for inst in insts:
                si = inst.sync_info
                if (si is not None and si.on_wait and len(si.on_wait) > _DRAIN_WAIT_LIMIT
                        and not str(inst.opcode).startswith("Tile")):
                    waits = list(si.on_wait)
                    chunks = [waits[i:i + _DRAIN_WAIT_LIMIT]
                              for i in range(0, len(waits), _DRAIN_WAIT_LIMIT)]
                    for chunk in chunks[:-1]:
                        nop = mybir.InstNoOp(
                            name=f"waitsplit-{counter[0]}", engine=inst.engine,
                            bass_nofuse=True,
                            sync_info=mybir.SyncInfo(on_wait=chunk, on_update=[]))
                        counter[0] += 1
                        new.append(nop)
                    inst.sync_info = mybir.SyncInfo(
                        on_wait=chunks[-1], on_update=list(si.on_update or []))
                new.append(inst)
            ordered[bbname] = new
        return orig_lower(self, ordered)

    tile.TileContext._lower_ordered_insts = _lower_ordered_insts

    def _drain_and_barrier(self, tick_clock, wait_clock):
        drain_inst = self.nc.sync.drain()
        wait_clock.add_sem_waits(drain_inst.ins, ScopedClock({None: tick_clock.global_clock}))
        si = drain_inst.ins.sync_info
        if si is not None and si.on_wait and len(si.on_wait) > _DRAIN_WAIT_LIMIT:
            waits = list(si.on_wait)
            drain_inst.ins.sync_info = mybir.SyncInfo(
                on_wait=waits[:_DRAIN_WAIT_LIMIT], on_update=list(si.on_update or []))
            for i in range(_DRAIN_WAIT_LIMIT, len(waits), _DRAIN_WAIT_LIMIT):
                extra = self.nc.sync.drain()
                extra.ins.sync_info = mybir.SyncInfo(
                    on_wait=waits[i:i + _DRAIN_WAIT_LIMIT], on_update=[])
        self.nc.all_engine_barrier()
        assert self.sems is not None
        popped = self.nc._tile_sem_poison_stack.pop()
        assert popped is self._sem_poison
        self.nc.clear_and_free_semaphores(list(self.sems.allocated().values()))
        self.nc.all_engine_barrier()

    tile.TileContext._drain_and_barrier = _drain_and_barrier
    tile.TileContext._drain_split_patched = True



def _patch_ldw_opt():
    """Flip walrus --enable-ldw-opt to true so back-to-back matmuls sharing a
    stationary operand skip the redundant LDWEIGHTS."""
    import concourse.bass_utils as bu
    if getattr(bu, "_ldw_opt_patched", False):
        return
    orig = bu.run_command

    def run_command_ldw(argv, **kwargs):
        argv = ["--enable-ldw-opt=true" if a == "--enable-ldw-opt=false" else a
                for a in argv]
        return orig(argv, **kwargs)

    bu.run_command = run_command_ldw
    bu._ldw_opt_patched = True


def _ns(j):
    """n-half slice."""
    return slice(j * 512, (j + 1) * 512)


def _cs(co):
    """128-wide channel-chunk slice."""
    return slice(co * 128, (co + 1) * 128)


def _emit(tc, aps):
    nc = tc.nc
    import contextlib

    ctx = contextlib.ExitStack()
    with ctx:
        cpool = ctx.enter_context(tc.tile_pool(name="consts", bufs=1))
        xpool = ctx.enter_context(tc.tile_pool(name="x", bufs=2))
        hpool = ctx.enter_context(tc.tile_pool(name="h", bufs=2))
        qpool = ctx.enter_context(tc.tile_pool(name="q", bufs=2))
        kpool = ctx.enter_context(tc.tile_pool(name="k", bufs=2))
        vtpool = ctx.enter_context(tc.tile_pool(name="vt", bufs=2))
        ptpool = ctx.enter_context(tc.tile_pool(name="pt", bufs=2))
        dpool = ctx.enter_context(tc.tile_pool(name="d", bufs=2))
        apool = ctx.enter_context(tc.tile_pool(name="attn", bufs=2))
        opool = ctx.enter_context(tc.tile_pool(name="osb", bufs=1))
        spool = ctx.enter_context(tc.tile_pool(name="stats", bufs=2))
        pmm = ctx.enter_context(tc.tile_pool(name="pmm", bufs=3, space="PSUM"))
        drpool = ctx.enter_context(tc.tile_pool(name="dscratch", bufs=2, space="DRAM"))

        # ---- input x first (it gates the GroupNorm stats critical path)
        x_tiles = []
        for b in range(B_LOC):
            x_t = xpool.tile([128, CO, N], F32, tag="x", name=f"x{b}")
            for co in range(CO):
                nc.sync.dma_start(out=x_t[:, co, :], in_=aps["x"][:, b, co])
            x_tiles.append(x_t)

        # ---- constants into SBUF
        wq_sb = cpool.tile([128, CO, C], BF16, tag="wq")
        wk_sb = cpool.tile([128, CO, C], BF16, tag="wk")
        wv_sb = cpool.tile([128, CO, C], BF16, tag="wv")
        wp_sb = cpool.tile([128, CO, C], BF16, tag="wp")
        for name, t in (("wqt", wq_sb), ("wkt", wk_sb), ("wvt", wv_sb), ("wptb", wp_sb)):
            nc.sync.dma_start(out=t, in_=aps[name])
        qb_sb = cpool.tile([128, CO], F32, tag="qb")
        kb_sb = cpool.tile([128, CO], F32, tag="kb")
        cb_sb = cpool.tile([128, CO], F32, tag="cb")
        nw_sb = cpool.tile([128, CO], F32, tag="nw")
        nb_sb = cpool.tile([128, CO], F32, tag="nb")
        for name, t in (("qb", qb_sb), ("kb", kb_sb), ("cb", cb_sb), ("nw", nw_sb), ("nbv", nb_sb)):
            nc.sync.dma_start(out=t, in_=aps[name])
        hind_sb = cpool.tile([128, 2], BF16, tag="hind")
        nc.sync.dma_start(out=hind_sb, in_=aps["hind"])
        hindT_sb = cpool.tile([2, 128], BF16, tag="hindT")
        nc.sync.dma_start(out=hindT_sb, in_=aps["hindT"])
        ones_sb = cpool.tile([128, 1], BF16, tag="ones1")
        nc.vector.memset(ones_sb, 1.0)
        eps_sb = cpool.tile([2, 1], F32, tag="eps")
        nc.vector.memset(eps_sb, EPS)

        mult = mybir.AluOpType.mult
        add = mybir.AluOpType.add
        sub = mybir.AluOpType.subtract
        AFT = mybir.ActivationFunctionType

        def emit_stats(b, x_t):
            # ---- GroupNorm stats: per-partition mean/var over N, then combine
            # over the 64-partition half that forms each group.
            mv = spool.tile([128, CO, 2], F32, tag="mv")
            for co in range(CO):
                st = spool.tile([128, 2, 6], F32, tag="bnst")
                xv = x_t[:, co, :].rearrange("p (s f) -> p s f", f=512)
                for sgrp in range(2):
                    nc.vector.bn_stats(out=st[:, sgrp, :], in_=xv[:, sgrp, :])
                nc.vector.bn_aggr(out=mv[:, co, :], in_=st)
            m2 = spool.tile([128, CO], F32, tag="m2")
            nc.vector.tensor_tensor(out=m2, in0=mv[:, :, 0], in1=mv[:, :, 0], op=mult)
            s8 = spool.tile([128, CO, 2], BF16, tag="s8")
            nc.vector.tensor_copy(out=s8[:, :, 0], in_=mv[:, :, 0])
            nc.vector.tensor_tensor(out=s8[:, :, 1], in0=mv[:, :, 1], in1=m2, op=add)
            gs_ps = pmm.tile([2, 2 * CO], F32, tag="mm")
            nc.tensor.matmul(gs_ps, lhsT=hind_sb, rhs=s8.rearrange("p a b -> p (a b)"),
                             start=True, stop=True)
            gmv = spool.tile([2, CO, 2], F32, tag="gmv")
            nc.vector.tensor_scalar_mul(gmv, gs_ps.rearrange("p (a b) -> p a b", b=2), 1.0 / 64.0)
            gm2 = spool.tile([2, CO], F32, tag="gm2")
            nc.vector.tensor_tensor(out=gm2, in0=gmv[:, :, 0], in1=gmv[:, :, 0], op=mult)
            gvar = spool.tile([2, CO], F32, tag="gvar")
            nc.vector.tensor_tensor(out=gvar, in0=gmv[:, :, 1], in1=gm2, op=sub)
            glog = spool.tile([2, CO], F32, tag="glog")
            nc.scalar.activation(glog, gvar, AFT.Ln, bias=eps_sb, scale=1.0)
            grstd = spool.tile([2, CO], F32, tag="grstd")
            nc.scalar.activation(grstd, glog, AFT.Exp, bias=0.0, scale=-0.5)
            gpack = spool.tile([2, CO, 2], BF16, tag="gpack")
            nc.vector.tensor_copy(out=gpack[:, :, 0], in_=gmv[:, :, 0])
            nc.vector.tensor_copy(out=gpack[:, :, 1], in_=grstd)
            bst_ps = pmm.tile([128, 2 * CO], F32, tag="mm")
            nc.tensor.matmul(bst_ps, lhsT=hindT_sb, rhs=gpack.rearrange("p a b -> p (a b)"),
                             start=True, stop=True)
            bs = spool.tile([128, CO, 2], F32, tag="bs")
            nc.vector.tensor_copy(out=bs, in_=bst_ps.rearrange("p (a b) -> p a b", b=2))
            # scale = rstd*w ; shift = mean - b/scale  => h = (x - shift)*scale
            scl = spool.tile([128, CO], F32, tag="scl")
            nc.vector.tensor_tensor(out=scl, in0=bs[:, :, 1], in1=nw_sb, op=mult)
            rscl = spool.tile([128, CO], F32, tag="rscl")
            nc.vector.reciprocal(rscl, scl)
            tmpb = spool.tile([128, CO], F32, tag="tmpb")
            nc.vector.tensor_tensor(out=tmpb, in0=nb_sb, in1=rscl, op=mult)
            shf = spool.tile([128, CO], F32, tag="shf")
            nc.vector.tensor_tensor(out=shf, in0=bs[:, :, 0], in1=tmpb, op=sub)
            h_t = hpool.tile([128, CO, N], BF16, tag="h")
            for co in range(CO):
                nc.vector.tensor_scalar(out=h_t[:, co, :], in0=x_t[:, co, :],
                                        scalar1=shf[:, co:co + 1], scalar2=scl[:, co:co + 1],
                                        op0=sub, op1=mult)
            return h_t

        def emit_qkv(b, h_t):
            # ---- q, k projections: q[c, n] accumulated over 4 k-tiles
            q_t = qpool.tile([128, CO, N], BF16, tag="q")
            k_t = kpool.tile([128, CO, N], BF16, tag="k")
            for wsb, bsb, dst in ((wq_sb, qb_sb, q_t), (wk_sb, kb_sb, k_t)):
                for co in range(CO):
                    ps = pmm.tile([128, N], F32, tag="mm")
                    for kt in range(CO):
                        for j in range(2):
                            nc.tensor.matmul(ps[:, _ns(j)], lhsT=wsb[:, kt, _cs(co)],
                                             rhs=h_t[:, kt, _ns(j)],
                                             start=(kt == 0), stop=(kt == CO - 1))
                    nc.scalar.activation(dst[:, co, :], ps, AFT.Identity,
                                         bias=bsb[:, co:co + 1], scale=1.0)

            # ---- vT = h^T @ Wv^T : [n, c] in bf16 (v bias folded into cb on host)
            vt = vtpool.tile([128, 8, C], BF16, tag="vt")
            for mp in range(4):
                ps = pmm.tile([128, N], F32, tag="mm")
                for j in range(2):
                    nchunk = mp * 2 + j
                    for kt in range(CO):
                        nc.tensor.matmul(ps[:, _ns(j)],
                                         lhsT=h_t[:, kt, nchunk * 128:(nchunk + 1) * 128],
                                         rhs=wv_sb[:, kt, :],
                                         start=(kt == 0), stop=(kt == CO - 1))
                nc.scalar.activation(vt[:, mp * 2:(mp + 1) * 2, :],
                                     ps.rearrange("p (a b) -> p a b", a=2), AFT.Copy)
            return q_t, k_t, vt

        def emit_heads(b, q_t, k_t, vt):
            # ---- attention per head (software-pipelined: scores of head hh+1
            # are emitted before the PV of head hh so the PE never stalls on exp)
            attn = apool.tile([128, NH, N], BF16, tag="attn")
            pts = {}
            # Shared denominator psum: head hh's two halves land on partition
            # row 32*hh (tile_position col offsets), so one reciprocal covers
            # the whole batch.
            dallB = pmm.tile([128, N], F32, tag="dallB", bufs=1)
            nc.vector.memset(dallB, 1.0)

            def emit_scores(hh):
                pt = ptpool.tile([128, 8, N], BF16, tag="pt")
                pts[hh] = pt
                for mc in range(8):
                    sps = pmm.tile([128, N], F32, tag="mm")
                    for j in range(2):
                        nc.tensor.matmul(sps[:, _ns(j)],
                                         lhsT=k_t[:, hh, mc * 128:(mc + 1) * 128],
                                         rhs=q_t[:, hh, _ns(j)],
                                         start=True, stop=True)
                    nc.scalar.activation(pt[:, mc, :], sps, AFT.Exp, scale=float(SCALE))

            def emit_pv(hh):
                pt = pts.pop(hh)
                # denominator = sum over all m: bf16 pairwise tree over the 8
                # chunk planes (DVE), then ones-matmul over the 128 partitions
                # into row 32*hh of the shared psum tile.
                tu = dpool.tile([128, 2, N], BF16, tag="dtu", bufs=1)
                tv = dpool.tile([128, 2, N], BF16, tag="dtv", bufs=1)
                dsum = dpool.tile([128, N], BF16, tag="dsum")
                nc.vector.tensor_tensor(out=tu, in0=pt[:, 0:2, :], in1=pt[:, 2:4, :], op=add)
                nc.vector.tensor_tensor(out=tv, in0=pt[:, 4:6, :], in1=pt[:, 6:8, :], op=add)
                nc.vector.tensor_tensor(out=tu, in0=tu, in1=tv, op=add)
                nc.vector.tensor_tensor(out=dsum, in0=tu[:, 0, :], in1=tu[:, 1, :], op=add)
                for j in range(2):
                    nc.tensor.matmul(dallB[32 * hh:32 * hh + 1, _ns(j)], lhsT=ones_sb,
                                     rhs=dsum[:, _ns(j)], start=True, stop=True,
                                     tile_position=(0, 32 * hh))
                # unnormalized PV -> attn (normalized in place once 1/denom is
                # broadcast, after all heads' denominators are in)
                pv = pmm.tile([128, N], F32, tag="mm")
                for mc in range(8):
                    for j in range(2):
                        nc.tensor.matmul(pv[:, _ns(j)], lhsT=vt[:, mc, hh * 128:(hh + 1) * 128],
                                         rhs=pt[:, mc, _ns(j)],
                                         start=(mc == 0), stop=(mc == 7))
                nc.scalar.activation(attn[:, hh, :], pv, AFT.Copy)

            emit_scores(0)
            for hh in range(1, NH):
                emit_scores(hh)
                emit_pv(hh - 1)
            emit_pv(NH - 1)

            # one 1/x = exp(-ln(x)) on ACT covers all 4 heads' denominators
            # (rows 32*hh); the memset-1.0 rows map to ln->0, exp->1. Ln and Exp
            # share the already-loaded ACT table set, and ACT is idle here.
            lnt = dpool.tile([128, N], F32, tag="lnt", bufs=1)
            nc.scalar.activation(lnt, dallB, AFT.Ln, bias=0.0, scale=1.0)
            rd128 = dpool.tile([128, N], F32, tag="rd128", bufs=1)
            nc.scalar.activation(rd128, lnt, AFT.Exp, bias=0.0, scale=-1.0)
            dn4 = drpool.tile([4, N], F32, tag="dn4")
            for hh in range(NH):
                nc.sync.dma_start(out=dn4[hh], in_=rd128[32 * hh:32 * hh + 1, :])
            for hh in range(NH):
                rdb = dpool.tile([128, N], F32, tag="rdb")
                row = dn4[hh]
                dn_bcast = bass.AP(tensor=row.tensor, offset=row.offset,
                                   ap=[[0, 128]] + list(row.ap))
                nc.sync.dma_start(out=rdb, in_=dn_bcast)
                nc.vector.tensor_tensor(out=attn[:, hh, :], in0=attn[:, hh, :],
                                        in1=rdb, op=mult)
            return attn

        def emit_proj(b, x_t, attn):
            # ---- proj + bias (cb = Wp@vb + pb) + residual
            osb = opool.tile([128, CO, N], F32, tag="osb")
            for co in range(CO):
                ps = pmm.tile([128, N], F32, tag="mm")
                for kt in range(CO):
                    for j in range(2):
                        nc.tensor.matmul(ps[:, _ns(j)], lhsT=wp_sb[:, kt, _cs(co)],
                                         rhs=attn[:, kt, _ns(j)],
                                         start=(kt == 0), stop=(kt == CO - 1))
                nc.vector.tensor_scalar(out=osb[:, co, :], in0=ps,
                                        scalar1=cb_sb[:, co:co + 1], scalar2=None,
                                        op0=add)
                nc.vector.tensor_tensor(out=osb[:, co, :], in0=osb[:, co, :],
                                        in1=x_t[:, co, :], op=add)
                nc.sync.dma_start(out=aps["out"][:, b, co], in_=osb[:, co, :])

        # interleaved schedule: batch 1's stats+qkv fill the PE trough left by
        # batch 0's denominator/normalize tail; proj(b0) overlaps heads(b1) ramp.
        x0, x1 = x_tiles
        h0 = emit_stats(0, x0)
        qkv0 = emit_qkv(0, h0)
        attn0 = emit_heads(0, *qkv0)
        h1 = emit_stats(1, x1)
        qkv1 = emit_qkv(1, h1)
        emit_proj(0, x0, attn0)
        attn1 = emit_heads(1, *qkv1)
        emit_proj(1, x1, attn1)


def build():
    """Build the per-core Bass program (same program on all 8 cores)."""
    _patch_tile_drain()
    nc = bass.Bass("TRN2", target_bir_lowering=False, debug=False)
    aps = {}
    aps["x"] = nc.dram_tensor("x", (128, B_LOC, CO, N), F32, kind="ExternalInput").ap()
    for name in ("wqt", "wkt", "wvt", "wptb"):
        aps[name] = nc.dram_tensor(name, (128, CO, C), BF16, kind="ExternalInput").ap()
    for name in ("qb", "kb", "cb", "nw", "nbv"):
        aps[name] = nc.dram_tensor(name, (128, CO), F32, kind="ExternalInput").ap()
    aps["hind"] = nc.dram_tensor("hind", (128, 2), BF16, kind="ExternalInput").ap()
    aps["hindT"] = nc.dram_tensor("hindT", (2, 128), BF16, kind="ExternalInput").ap()
    aps["out"] = nc.dram_tensor("out", (128, B_LOC, CO, N), F32, kind="ExternalOutput").ap()
    with tile.TileContext(nc) as tc:
        _emit(tc, aps)
    return nc


def _tile_w(wt):
    """[C_in, C_out] -> [128, CO(kt), C_out] partition-tiled, contiguous."""
    return np.ascontiguousarray(wt.reshape(CO, 128, C).transpose(1, 0, 2))


def _tile_v(v):
    """[C] -> [128, CO] with c = co*128 + p."""
    return np.ascontiguousarray(np.asarray(v, np.float32).reshape(CO, 128).T)


def make_in_maps(x, norm_w, norm_b, q_w, q_b, k_w, k_b, v_w, v_b, p_w, p_b):
    """Host-side prep: shard x over 8 cores, pre-transpose/tile weights, fold biases."""
    f = lambda a: np.ascontiguousarray(np.asarray(a, dtype=np.float32))
    x = f(x).reshape(B, C, N)
    wqt = _tile_w(f(q_w).T.astype(ml_dtypes.bfloat16))
    wkt = _tile_w(f(k_w).T.astype(ml_dtypes.bfloat16))
    wvt = _tile_w(f(v_w).T.astype(ml_dtypes.bfloat16))
    wptb = _tile_w(f(p_w).T.astype(ml_dtypes.bfloat16))
    cb = _tile_v(f(p_w) @ f(v_b) + f(p_b))
    hind = np.zeros((128, 2), ml_dtypes.bfloat16)
    hind[:64, 0] = 1.0
    hind[64:, 1] = 1.0
    hindT = np.ascontiguousarray(hind.T)
    shared = dict(wqt=wqt, wkt=wkt, wvt=wvt, wptb=wptb, qb=_tile_v(q_b), kb=_tile_v(k_b),
                  cb=cb, nw=_tile_v(norm_w), nbv=_tile_v(norm_b), hind=hind, hindT=hindT)
    in_maps = []
    for c in range(N_CORES):
        m = dict(shared)
        # [B_LOC, C, N] -> [128, B_LOC, CO, N]
        xs = x[c * B_LOC:(c + 1) * B_LOC].reshape(B_LOC, CO, 128, N)
        m["x"] = np.ascontiguousarray(xs.transpose(2, 0, 1, 3))
        in_maps.append(m)
    return in_maps


_last_results = None  # test.py reads this for profile info


def kernel(**inputs) -> np.ndarray:
    global _BUILT, _last_results
    from concourse.bass_utils import run_bass_kernel_spmd

    if _BUILT is None:
        _BUILT = build()
    nc = _BUILT
    in_maps = make_in_maps(**inputs)
    res = run_bass_kernel_spmd(nc, in_maps, core_ids=list(range(N_CORES)))
    _last_results = res
    # per-core out is [128, B_LOC, CO, N] -> [B_LOC, C, N]
    outs = [r["out"].transpose(1, 2, 0, 3).reshape(B_LOC, C, N) for r in res.results]
    out = np.concatenate(outs, axis=0)
    return out.reshape(B, C, HW, HW).astype(np.float32)
